# revision 1
# baseline (speedup 1.0000x reference)
"""Causal self-attention (GQA, RoPE) Trainium2 Bass kernel.

Full inputs in, full output out. Tensor-parallel over heads across 8
NeuronCores: core i computes q-heads 4i..4i+3 (kv head i) and a partial
output projection over its 256 attn-out features; the host sums the 8
partial outputs (the "all-reduce after output_proj" step).
"""

import numpy as np

import concourse.bacc as bacc
import concourse.mybir as mybir
import concourse.tile as tile
from concourse.bass_utils import run_bass_kernel_spmd

S = 2048          # sequence length
E = 2048          # embedding dim
H = 32            # query heads
KV = 8            # kv heads
HD = 64           # head dim
NCORES = 8
HC = H // NCORES  # query heads per core = 4
DQ = HC * HD      # per-core q proj width = 256
DKV = HD          # per-core kv proj width = 64
DQK = DQ + DKV    # roped span = 320
DW = DQ + 2 * DKV  # fused qkv proj width = 384
ST = S // 128     # 16 s-tiles of 128 rows
MASK_NEG = -1.0e4  # pre-scale additive mask (scaled: -1250 -> exp == 0)

F32 = mybir.dt.float32
F32R = mybir.dt.float32r


def r(ap):
    """Bitcast an AP to float32r so the PE runs fast-mode fp32 matmuls."""
    return ap.bitcast(F32R)


def build_nc(seq_tiles=ST, reps=1, phases=(1, 2, 3)):
    """Build + compile the per-core Bass program (identical on all cores)."""
    st_n = seq_tiles
    s_n = st_n * 128
    qb_n = s_n // 512

    nc = bacc.Bacc("TRN2", target_bir_lowering=False, debug=False)
    x_d = nc.dram_tensor("x", [s_n, E], F32R, kind="ExternalInput")
    wt_d = nc.dram_tensor("wt", [E, DW], F32R, kind="ExternalInput")
    wot_d = nc.dram_tensor("wot", [DQ, E], F32R, kind="ExternalInput")
    cos_d = nc.dram_tensor("cosh", [s_n, DQK // 2], F32, kind="ExternalInput")
    sin_d = nc.dram_tensor("sinh", [s_n, DQK // 2], F32, kind="ExternalInput")
    mask_d = nc.dram_tensor("maskadd", [512, 512], F32, kind="ExternalInput")
    id_d = nc.dram_tensor("ident", [128, 128], F32R, kind="ExternalInput")
    out_d = nc.dram_tensor("out", [s_n, E], F32, kind="ExternalOutput")

    with tile.TileContext(nc) as tc:
        for _rep in range(reps):
            # ---------- persistent constants / cross-phase tensors ----------
            with (
                tc.tile_pool(name="const", bufs=1) as constp,
                tc.tile_pool(name="qkv_store", bufs=1) as storep,
            ):
                ident = constp.tile([128, 128], F32R)
                nc.sync.dma_start(out=ident[:], in_=id_d.ap()[:, :])

                woT_sb = constp.tile([128, 2, E], F32R)
                nc.sync.dma_start(
                    out=woT_sb[:], in_=wot_d.ap().rearrange("(c p) e -> p c e", p=128)
                )
                mask_sb = constp.tile([128, 4, 512], F32)
                nc.sync.dma_start(
                    out=mask_sb[:], in_=mask_d.ap().rearrange("(r p) k -> p r k", p=128)
                )

                # qT: all heads on partitions 0:64; head h of s-tile t in
                # cols t*512 + h*128.
                qT_sb = storep.tile([64, st_n * 512], F32R)
                # kT: kv head on partitions 0:64.
                kT_sb = storep.tile([64, s_n], F32R)
                # v: [s, d] duplicated along free (cols 2*64 per s-tile) so the
                # AV matmul writes the full 128 psum partitions.
                v_sb = storep.tile([128, st_n * 2 * DKV], F32R)
                # attn-out transposed: head-pair hp in col block hp*s_n.
                aoT_sb = storep.tile([128, 2 * s_n], F32R)

                # ================= phase 1: qkv proj + rope =================
                with (
                    tc.tile_pool(name="p1_sbuf", bufs=2) as p1,
                    tc.tile_pool(name="p1_w", bufs=1) as p1w,
                    tc.tile_pool(name="p1_xt", bufs=3) as p1x,
                    tc.tile_pool(name="p1_ps_xt", bufs=2, space="PSUM") as ps_xt_p,
                    tc.tile_pool(name="p1_ps_qkv", bufs=2, space="PSUM") as ps_qkv_p,
                    tc.tile_pool(name="p1_ps_tr", bufs=2, space="PSUM") as ps_tr_p,
                ):
                    wT_sb = p1w.tile([128, E // 128, DW], F32R)
                    for j in range(E // 128):
                        nc.sync.dma_start(
                            out=wT_sb[:, j, :],
                            in_=wt_d.ap()[j * 128:(j + 1) * 128, :],
                        )

                    for t in range(st_n if 1 in phases else 0):
                        x_sb = p1.tile([128, E], F32R, tag="x")
                        nc.sync.dma_start(
                            out=x_sb[:], in_=x_d.ap()[t * 128:(t + 1) * 128, :]
                        )
                        cs_sb = p1.tile([128, 2, DQK // 2], F32, tag="cs")
                        nc.sync.dma_start(
                            out=cs_sb[:, 0, :], in_=cos_d.ap()[t * 128:(t + 1) * 128, :]
                        )
                        nc.sync.dma_start(
                            out=cs_sb[:, 1, :], in_=sin_d.ap()[t * 128:(t + 1) * 128, :]
                        )
                        ps_qkv = ps_qkv_p.tile([128, DW], F32, tag="qkv")
                        for jg in range(E // 512):
                            ps_xt = ps_xt_p.tile([128, 512], F32, tag="xt")
                            for m in range(4):
                                nc.tensor.matmul(
                                    r(ps_xt[:, m * 128:(m + 1) * 128]),
                                    r(x_sb[:, (4 * jg + m) * 128:(4 * jg + m + 1) * 128]),
                                    r(ident[:]),
                                    is_transpose=True,
                                    start=(m == 0),
                                    stop=(m == 3),
                                )
                            xt_sb = p1x.tile([128, 512], F32R, tag="xts")
                            nc.vector.tensor_copy(xt_sb[:], ps_xt[:])
                            for m in range(4):
                                j = 4 * jg + m
                                nc.tensor.matmul(
                                    ps_qkv[:],
                                    r(xt_sb[:, m * 128:(m + 1) * 128]),
                                    r(wT_sb[:, j, :]),
                                    start=(j == 0),
                                    stop=(j == E // 128 - 1),
                                )

                        # ---- rope on q+k jointly (320 cols); copy v ----
                        pairs = DQK // 2  # 160
                        qk_sb = p1.tile([128, DQK], F32R, tag="qkro")
                        se = ps_qkv[:, 0:DQK].rearrange("p (n two) -> p two n", two=2)
                        de = qk_sb[:].rearrange("p (n two) -> p two n", two=2)
                        c_ap = cs_sb[:, 0, :]
                        s_ap = cs_sb[:, 1, :]
                        t1 = p1.tile([128, pairs], F32, tag="t1")
                        t2 = p1.tile([128, pairs], F32, tag="t2")
                        nc.vector.tensor_mul(t1[:], se[:, 0, :], c_ap)
                        nc.vector.tensor_mul(t2[:], se[:, 1, :], s_ap)
                        nc.vector.tensor_sub(de[:, 0, :], t1[:], t2[:])
                        t3 = p1.tile([128, pairs], F32, tag="t3")
                        t4 = p1.tile([128, pairs], F32, tag="t4")
                        nc.vector.tensor_mul(t3[:], se[:, 1, :], c_ap)
                        nc.vector.tensor_mul(t4[:], se[:, 0, :], s_ap)
                        nc.vector.tensor_add(de[:, 1, :], t3[:], t4[:])

                        for dup in range(2):
                            nc.vector.tensor_copy(
                                v_sb[:, t * 2 * DKV + dup * DKV:t * 2 * DKV + (dup + 1) * DKV],
                                ps_qkv[:, DQK:DW],
                            )

                        # ---- transpose roped q/k into qT/kT (partitions 0:64) ----
                        ps_trq = ps_tr_p.tile([64, 512], F32, tag="trq")
                        for hh in range(4):
                            nc.tensor.matmul(
                                r(ps_trq[:, hh * 128:(hh + 1) * 128]),
                                r(qk_sb[:, hh * 64:(hh + 1) * 64]),
                                r(ident[:]),
                                is_transpose=True,
                                start=(hh == 0),
                                stop=(hh == 3),
                            )
                        nc.vector.tensor_copy(
                            qT_sb[:, t * 512:(t + 1) * 512], ps_trq[:]
                        )
                        ps_trk = ps_tr_p.tile([64, 128], F32, tag="trk")
                        nc.tensor.matmul(
                            r(ps_trk[:]), r(qk_sb[:, 256:DQK]), r(ident[:]),
                            is_transpose=True, start=True, stop=True,
                        )
                        nc.vector.tensor_copy(
                            kT_sb[:, t * 128:(t + 1) * 128], ps_trk[:]
                        )

                # ================= phase 2: attention =================
                with (
                    tc.tile_pool(name="p2_a", bufs=3) as p2a,
                    tc.tile_pool(name="p2_at", bufs=2) as p2t,
                    tc.tile_pool(name="p2_small", bufs=12) as p2s,
                    tc.tile_pool(name="p3_o", bufs=2) as p3o,
                    tc.tile_pool(name="p2_ps_s", bufs=3, space="PSUM") as ps_s_p,
                    tc.tile_pool(name="p2_ps_at", bufs=3, space="PSUM") as ps_at_p,
                    tc.tile_pool(name="p2_ps_av", bufs=1, space="PSUM") as ps_av_p,
                    tc.tile_pool(name="p3_ps", bufs=1, space="PSUM") as ps_o_p,
                ):
                    for qb in range(qb_n if 2 in phases else 0):
                        for h in range(HC):
                            p0 = 64 * (h & 1)
                            hp2 = h >> 1
                            nch = 4 * qb + 4  # causal 128-chunks for this q block
                            atT = p2t.tile([128, st_n * 512], F32R, tag="atT")
                            atv = atT[:].rearrange("p (kc f) -> p kc f", f=512)
                            for qs in range(4):
                                qt = 4 * qb + qs
                                nblk = qb + 1  # 512-wide k blocks
                                a_sb = p2a.tile([128, 2048], F32R, tag="a")
                                rs_all = p2s.tile([128, 4], F32, tag="rs")
                                for kb in range(nblk):
                                    ps_s = ps_s_p.tile([128, 512], F32, tag="s")
                                    nc.tensor.matmul(
                                        ps_s[:],
                                        r(qT_sb[:, qt * 512 + h * 128:qt * 512 + (h + 1) * 128]),
                                        r(kT_sb[:, kb * 512:(kb + 1) * 512]),
                                        start=True,
                                        stop=True,
                                    )
                                    if kb == qb:  # diagonal block: additive mask
                                        nc.vector.tensor_add(
                                            ps_s[:], ps_s[:], mask_sb[:, qs, :]
                                        )
                                    nc.scalar.activation(
                                        a_sb[:, kb * 512:(kb + 1) * 512],
                                        ps_s[:],
                                        mybir.ActivationFunctionType.Exp,
                                        scale=0.125,
                                        accum_out=rs_all[:, kb:kb + 1],
                                    )
                                tot = p2s.tile([128, 1], F32, tag="rtot")
                                nc.vector.reduce_sum(
                                    tot[:], rs_all[:, 0:nblk], axis=mybir.AxisListType.X
                                )
                                rinv = p2s.tile([128, 1], F32, tag="rinv")
                                nc.vector.reciprocal(rinv[:], tot[:])
                                for kb in range(nblk):
                                    nc.vector.tensor_scalar_mul(
                                        a_sb[:, kb * 512:(kb + 1) * 512],
                                        a_sb[:, kb * 512:(kb + 1) * 512],
                                        rinv[:],
                                    )
                                # transpose causal chunks kc <= qt into atT
                                for kg in range((qt + 4) // 4):
                                    cnt = min(4, qt + 1 - 4 * kg)
                                    ps_at = ps_at_p.tile([128, 512], F32, tag="at")
                                    for m in range(cnt):
                                        kc = 4 * kg + m
                                        nc.tensor.matmul(
                                            r(ps_at[:, m * 128:(m + 1) * 128]),
                                            r(a_sb[:, kc * 128:(kc + 1) * 128]),
                                            r(ident[:]),
                                            is_transpose=True,
                                            start=(m == 0),
                                            stop=(m == cnt - 1),
                                        )
                                    nc.vector.tensor_copy(
                                        atv[:, 4 * kg:4 * kg + cnt,
                                            qs * 128:(qs + 1) * 128],
                                        ps_at[:, 0:cnt * 128].rearrange(
                                            "p (a b) -> p a b", b=128
                                        ),
                                    )
                            # ---- AV: outT[d, q512] accumulated over k chunks ----
                            ps_av = ps_av_p.tile([128, 512], F32, tag="av")
                            for kc in range(nch):
                                # chunks past the diagonal have no attn mass for
                                # early q subtiles; skip those columns entirely
                                lo = max(0, kc - 4 * qb) * 128
                                nc.tensor.matmul(
                                    ps_av[:, lo:512],
                                    r(v_sb[:, kc * 2 * DKV:(kc + 1) * 2 * DKV]),
                                    r(atv[:, kc, lo:512]),
                                    start=(kc == 0),
                                    stop=(kc == nch - 1),
                                )
                            nc.vector.tensor_copy(
                                aoT_sb[p0:p0 + 64, hp2 * s_n + qb * 512:hp2 * s_n + (qb + 1) * 512],
                                ps_av[p0:p0 + 64, :],
                            )

                        # ---- phase 3 for this q block: output projection ----
                        for st in range(4 * qb, (4 * qb + 4) if 3 in phases else 4 * qb):
                            o_sb = p3o.tile([128, E], F32, tag="o")
                            for eb in range(E // 512):
                                ps_o = ps_o_p.tile([128, 512], F32, tag="po")
                                for c in range(2):
                                    nc.tensor.matmul(
                                        ps_o[:],
                                        r(aoT_sb[:, c * s_n + st * 128:c * s_n + (st + 1) * 128]),
                                        r(woT_sb[:, c, eb * 512:(eb + 1) * 512]),
                                        start=(c == 0),
                                        stop=(c == 1),
                                    )
                                nc.scalar.copy(o_sb[:, eb * 512:(eb + 1) * 512], ps_o[:])
                            nc.sync.dma_start(
                                out=out_d.ap()[st * 128:(st + 1) * 128, :], in_=o_sb[:]
                            )

    nc.compile()
    return nc


def make_tables(s_n=S):
    """Host-side RoPE tables and additive causal mask."""
    theta = (1.0 / (10000.0 ** (np.arange(0, HD, 2, dtype=np.float32) / HD))).astype(
        np.float32
    )
    freqs = np.arange(s_n, dtype=np.float32)[:, None] * theta[None, :]  # [s, 32]
    cos = np.cos(freqs).astype(np.float32)
    sin = np.sin(freqs).astype(np.float32)
    cosh = np.tile(cos, (1, DQK // HD))  # [s, 160]
    sinh = np.tile(sin, (1, DQK // HD))
    a = np.arange(512)
    maskadd = np.where(a[None, :] <= a[:, None], 0.0, MASK_NEG).astype(np.float32)
    return cosh, sinh, maskadd


def make_core_inputs(x2, wq, wk, wv, wo, core):
    """Per-core input dict (host-side sharding prep)."""
    cosh, sinh, maskadd = _TABLES
    i = core
    wq_i = wq[i * DQ:(i + 1) * DQ]
    wk_i = wk[i * DKV:(i + 1) * DKV]
    wv_i = wv[i * DKV:(i + 1) * DKV]
    wt = np.ascontiguousarray(np.concatenate([wq_i, wk_i, wv_i], axis=0).T)
    wot = np.ascontiguousarray(wo[:, i * DQ:(i + 1) * DQ].T)
    return {
        "x": x2,
        "wt": wt.astype(np.float32),
        "wot": wot.astype(np.float32),
        "cosh": cosh,
        "sinh": sinh,
        "maskadd": maskadd,
        "ident": np.eye(128, dtype=np.float32),
    }


_TABLES = make_tables()
_NC_CACHE = {}


def _get_nc(reps=1):
    key = ("nc", reps)
    if key not in _NC_CACHE:
        _NC_CACHE[key] = build_nc(reps=reps)
    return _NC_CACHE[key]


def kernel(x, wq, wk, wv, wo):
    x = np.asarray(x, dtype=np.float32)
    b, s_n, e = x.shape
    x2 = np.ascontiguousarray(x.reshape(s_n, e))
    in_maps = [
        make_core_inputs(x2, np.asarray(wq, np.float32), np.asarray(wk, np.float32),
                         np.asarray(wv, np.float32), np.asarray(wo, np.float32), i)
        for i in range(NCORES)
    ]
    res = run_bass_kernel_spmd(_get_nc(), in_maps, core_ids=list(range(NCORES)))
    out = np.zeros((s_n, e), dtype=np.float32)
    for rr in res.results:
        out += rr["out"]
    return out.reshape(b, s_n, e).astype(np.float32)



# revision 6
# speedup vs baseline: 1.6009x; 1.6009x over previous
"""Causal self-attention (GQA, RoPE) Trainium2 Bass kernel.

Full inputs in, full output out. Tensor-parallel over heads across 8
NeuronCores: core i computes q-heads 4i..4i+3 (kv head i) and a partial
output projection over its 256 attn-out features; the host sums the 8
partial outputs (the "all-reduce after output_proj" step).

v2 design notes (vs the v1 kernel):
- x is transposed on the HOST and shipped as bf16 [E, S]; the qkv
  projection consumes it directly (no on-device x transposes).
- Attention is computed in transposed layout: sT[k,q] = kT.T @ qT and
  outT[d,q] = v.T @ exp(sT), so no per-chunk attention transposes and
  no PSUM-evacuate/reload chains. Softmax denominators come from a
  ones-column appended to v; normalization is one rank-1 broadcast
  matmul + one DVE multiply per (head, q-block).
- Per-head features are host-permuted to [evens, odds] so RoPE is
  contiguous elementwise in both layouts; v/wo stay unpermuted.
- All matmul operands are bf16 (FWL fast weight loads); PSUM stays
  fp32.
"""

import numpy as np

import concourse.bacc as bacc
import concourse.mybir as mybir
import concourse.tile as tile
from concourse.bass_utils import run_bass_kernel_spmd

S = 2048          # sequence length
E = 2048          # embedding dim
H = 32            # query heads
KV = 8            # kv heads
HD = 64           # head dim
NCORES = 8
HC = H // NCORES  # query heads per core = 4
DQ = HC * HD      # per-core q proj width = 256
DKV = HD          # per-core kv proj width = 64
DQK = DQ + DKV    # roped span = 320
DW = DQ + 2 * DKV  # fused qkv proj width = 384
ST = S // 128     # 16 s-tiles of 128 rows
MASK_NEG = -1.0e4  # pre-scale additive mask (scaled: -1250 -> exp == 0)

F32 = mybir.dt.float32
F32R = mybir.dt.float32r
BF16 = mybir.dt.bfloat16


def r(ap):
    """Bitcast an AP to float32r so the PE runs fast-mode fp32 matmuls."""
    return ap.bitcast(F32R)


def build_nc(seq_tiles=ST, reps=1, phases=(1, 2, 3)):
    """Build + compile the per-core Bass program (identical on all cores)."""
    st_n = seq_tiles
    s_n = st_n * 128
    nit = st_n  # (head, q-block) iterations: 4 heads x st_n/4 q-blocks

    nc = bacc.Bacc("TRN2", target_bir_lowering=False, debug=False)
    xt_d = nc.dram_tensor("xt", [E, s_n], BF16, kind="ExternalInput")
    wt_d = nc.dram_tensor("wt", [E, DW], BF16, kind="ExternalInput")
    wot_d = nc.dram_tensor("wot", [DQ, E], BF16, kind="ExternalInput")
    cos_d = nc.dram_tensor("cosh", [s_n, DQK // 2], F32, kind="ExternalInput")
    sin_d = nc.dram_tensor("sinh", [s_n, DQK // 2], F32, kind="ExternalInput")
    mask_d = nc.dram_tensor("maskadd", [128, 128], F32, kind="ExternalInput")
    id_d = nc.dram_tensor("ident", [128, 128], BF16, kind="ExternalInput")
    out_d = nc.dram_tensor("out", [s_n, E], F32, kind="ExternalOutput")

    with tile.TileContext(nc) as tc:
        for _rep in range(reps):
            with (
                tc.tile_pool(name="const", bufs=1) as constp,
                tc.tile_pool(name="store", bufs=1) as storep,
            ):
                ident = constp.tile([128, 128], BF16)
                nc.sync.dma_start(out=ident[:], in_=id_d.ap()[:, :])
                maskT_sb = constp.tile([128, 128], F32)
                nc.sync.dma_start(out=maskT_sb[:], in_=mask_d.ap()[:, :])
                wT_sb = constp.tile([128, E // 128, DW], BF16)
                nc.sync.dma_start(
                    out=wT_sb[:], in_=wt_d.ap().rearrange("(c p) f -> p c f", p=128)
                )
                woT_sb = constp.tile([128, 2, E], BF16)
                nc.sync.dma_start(
                    out=woT_sb[:], in_=wot_d.ap().rearrange("(c p) e -> p c e", p=128)
                )
                ones_sb = constp.tile([128, 128], F32)
                nc.vector.memset(ones_sb[:], 1.0)

                # qT: head h cols [h, s]; kT: [s]; d on partitions 0:64.
                qT_sb = storep.tile([64, HC, s_n], BF16)
                kT_sb = storep.tile([64, s_n], BF16)
                # v in [s, d] per 128-chunk; _ev has a ones col at 64 (den row
                # 64 of AV psum), _od has ones col at 0 + v at 64:128.
                v_ev = storep.tile([128, st_n, 72], BF16)
                v_od = storep.tile([128, st_n, 128], BF16)
                nc.vector.memset(v_ev[:, :, 64:72], 1.0)
                nc.vector.memset(v_od[:, :, 0:1], 1.0)
                nc.vector.memset(v_od[:, :, 1:64], 0.0)
                # attn-out transposed: feature d = c*128 + p, col = s.
                aoT_sb = storep.tile([128, 2, s_n], BF16)

                # ================= phase 1: qkv proj + rope =================
                with (
                    tc.tile_pool(name="p1_x", bufs=2) as p1x,
                    tc.tile_pool(name="p1_sb", bufs=2) as p1,
                    tc.tile_pool(name="p1_ps_qkv", bufs=2, space="PSUM") as ps_qkv_p,
                    tc.tile_pool(name="p1_ps_tr", bufs=2, space="PSUM") as ps_tr_p,
                ):
                    xt_r = xt_d.ap().rearrange("(c p) s -> p c s", p=128)
                    xT_blk = [None] * (st_n // 4)
                    prev = None  # (qk_sb tile, ps_qkv, t)
                    tr_prev = None  # (ps_tr, t)

                    def emit_tr(qk_sb, t):
                        ps_tr = ps_tr_p.tile([64, 5, 128], BF16, tag="tr")
                        for g in range(5):
                            nc.tensor.matmul(
                                ps_tr[:, g, :],
                                qk_sb[:, g * 64:(g + 1) * 64],
                                ident[:],
                                is_transpose=True,
                                start=(g == 0),
                                stop=(g == 4),
                            )
                        return ps_tr

                    def emit_trcopy(ps_tr, t):
                        nc.vector.tensor_copy(
                            qT_sb[:, :, t * 128:(t + 1) * 128], ps_tr[:, 0:4, :]
                        )
                        nc.vector.tensor_copy(
                            kT_sb[:, t * 128:(t + 1) * 128], ps_tr[:, 4, :]
                        )

                    for t in range(st_n if 1 in phases else 0):
                        blk, ts = t // 4, t % 4
                        if ts == 0:
                            xT_blk[blk] = p1x.tile(
                                [128, E // 128, 512], BF16, tag="xT", name="xTb"
                            )
                            nc.sync.dma_start(
                                out=xT_blk[blk][:],
                                in_=xt_r[:, :, blk * 512:(blk + 1) * 512],
                            )
                        cs_sb = p1.tile([128, 2, DQK // 2], F32, tag="cs")
                        nc.sync.dma_start(
                            out=cs_sb[:, 0, :], in_=cos_d.ap()[t * 128:(t + 1) * 128, :]
                        )
                        nc.sync.dma_start(
                            out=cs_sb[:, 1, :], in_=sin_d.ap()[t * 128:(t + 1) * 128, :]
                        )
                        ps_qkv = ps_qkv_p.tile([128, DW], F32, tag="qkv")
                        for c in range(E // 128):
                            nc.tensor.matmul(
                                ps_qkv[:],
                                xT_blk[blk][:, c, ts * 128:(ts + 1) * 128],
                                wT_sb[:, c, :],
                                start=(c == 0),
                                stop=(c == E // 128 - 1),
                            )
                        if tr_prev is not None:
                            emit_trcopy(*tr_prev)
                            tr_prev = None
                        if prev is not None:
                            tr_prev = (emit_tr(*prev), prev[1])

                        # ---- rope on q+k jointly (5 groups of [ev|od]) ----
                        se = ps_qkv[:, 0:DQK].rearrange(
                            "p (g two d) -> p two g d", two=2, d=32
                        )
                        qk_sb = p1.tile([128, DQK], BF16, tag="qkro")
                        de = qk_sb[:].rearrange("p (g two d) -> p two g d", two=2, d=32)
                        c_ap = cs_sb[:, 0, :].rearrange("p (g d) -> p g d", d=32)
                        s_ap = cs_sb[:, 1, :].rearrange("p (g d) -> p g d", d=32)
                        t1 = p1.tile([128, DQK // 2], F32, tag="t1")
                        t2 = p1.tile([128, DQK // 2], F32, tag="t2")
                        nc.vector.tensor_mul(t1[:], se[:, 0, :, :], c_ap)
                        nc.vector.tensor_mul(t2[:], se[:, 1, :, :], s_ap)
                        nc.vector.tensor_sub(de[:, 0, :, :], t1[:], t2[:])
                        t3 = p1.tile([128, DQK // 2], F32, tag="t3")
                        t4 = p1.tile([128, DQK // 2], F32, tag="t4")
                        nc.vector.tensor_mul(t3[:], se[:, 1, :, :], c_ap)
                        nc.vector.tensor_mul(t4[:], se[:, 0, :, :], s_ap)
                        nc.vector.tensor_add(de[:, 1, :, :], t3[:], t4[:])
                        # ---- v copies (both AV lhsT layouts) ----
                        nc.vector.tensor_copy(v_ev[:, t, 0:64], ps_qkv[:, DQK:DW])
                        nc.vector.tensor_copy(v_od[:, t, 64:128], ps_qkv[:, DQK:DW])
                        prev = (qk_sb, t)

                    if tr_prev is not None:
                        emit_trcopy(*tr_prev)
                    if prev is not None:
                        ps_tr = emit_tr(*prev)
                        emit_trcopy(ps_tr, prev[1])

                # ============ phase 2+3: attention + output proj ============
                with (
                    tc.tile_pool(name="p2_aT", bufs=2) as p2a,
                    tc.tile_pool(name="p2_rt", bufs=2) as p2r,
                    tc.tile_pool(name="p3_o", bufs=2) as p3o,
                    tc.tile_pool(name="p2_ps_s", bufs=3, space="PSUM") as ps_s_p,
                    tc.tile_pool(name="p2_ps_av", bufs=2, space="PSUM") as ps_av_p,
                    tc.tile_pool(name="p2_ps_bc", bufs=1, space="PSUM") as ps_bc_p,
                    tc.tile_pool(name="p3_ps", bufs=2, space="PSUM") as ps_o_p,
                ):
                    qb_n = st_n // 4

                    def emit_phase3(qb):
                        for st in range(4 * qb, 4 * qb + 4):
                            o_sb = p3o.tile([128, E], F32, tag="o")
                            for eb in range(E // 512):
                                ps_o = ps_o_p.tile([128, 512], F32, tag="po")
                                for c2 in range(2):
                                    nc.tensor.matmul(
                                        ps_o[:],
                                        aoT_sb[:, c2, st * 128:(st + 1) * 128],
                                        woT_sb[:, c2, eb * 512:(eb + 1) * 512],
                                        start=(c2 == 0),
                                        stop=(c2 == 1),
                                    )
                                nc.vector.tensor_copy(
                                    o_sb[:, eb * 512:(eb + 1) * 512], ps_o[:]
                                )
                            nc.sync.dma_start(
                                out=out_d.ap()[st * 128:(st + 1) * 128, :], in_=o_sb[:]
                            )

                    aT_t = {}
                    av_t = {}
                    for it in range(nit + 1 if 2 in phases else 0):
                        cur = it if it < nit else None
                        prv = it - 1 if it > 0 else None
                        if cur is not None:
                            qb, h = divmod(cur, 4)
                            k_cur = 4 * qb + 4
                            aT = p2a.tile([128, st_n, 512], BF16, tag="aT")
                            aT_t[cur] = aT
                        if prv is not None:
                            pq, ph = divmod(prv, 4)
                            k_prv = 4 * pq + 4
                            ps_av = ps_av_p.tile([128, 512], F32, tag="av")
                            av_t[prv] = ps_av
                            pT = aT_t[prv]
                        nk = max(k_cur if cur is not None else 0,
                                 k_prv if prv is not None else 0)
                        for kc in range(nk):
                            if cur is not None and kc < k_cur:
                                lo = 128 * max(0, kc - 4 * qb)
                                ps_s = ps_s_p.tile([128, 512], F32, tag="s")
                                nc.tensor.matmul(
                                    ps_s[:, lo:512],
                                    kT_sb[:, kc * 128:(kc + 1) * 128],
                                    qT_sb[:, h, qb * 512 + lo:(qb + 1) * 512],
                                    start=True,
                                    stop=True,
                                )
                                if kc >= 4 * qb:  # diagonal chunk: causal mask
                                    nc.vector.tensor_add(
                                        ps_s[:, lo:lo + 128],
                                        ps_s[:, lo:lo + 128],
                                        maskT_sb[:],
                                    )
                                nc.scalar.activation(
                                    aT[:, kc, lo:512],
                                    ps_s[:, lo:512],
                                    mybir.ActivationFunctionType.Exp,
                                    scale=0.125,
                                )
                            if prv is not None and kc < k_prv:
                                lo = 128 * max(0, kc - 4 * pq)
                                vt = (v_ev[:, kc, 0:65] if ph % 2 == 0
                                      else v_od[:, kc, :])
                                np_out = 65 if ph % 2 == 0 else 128
                                nc.tensor.matmul(
                                    ps_av[0:np_out, lo:512],
                                    vt,
                                    pT[:, kc, lo:512],
                                    start=(kc == 0),
                                    stop=(kc == k_prv - 1),
                                )
                        if prv is not None:
                            # normalize: aoT = v-out rows * (1/den) bcast
                            dr = 64 if ph % 2 == 0 else 0
                            rtmp = p2r.tile([128, 512], F32R, tag="rt")
                            with nc.allow_low_precision(reason="rinv rounds to f32r"):
                                nc.vector.reciprocal(
                                    rtmp[dr:dr + 1, :], ps_av[dr:dr + 1, :]
                                )
                            ps_bc = ps_bc_p.tile([128, 512], F32, tag="bc")
                            nc.tensor.matmul(
                                ps_bc[:],
                                r(ones_sb[dr:dr + 1, :]),
                                rtmp[dr:dr + 1, :],
                                start=True,
                                stop=True,
                            )
                            p0 = 64 * (ph & 1)
                            hp = ph >> 1
                            dst = aoT_sb[p0:p0 + 64, hp, pq * 512:(pq + 1) * 512]
                            nc.vector.tensor_copy(dst, ps_av[p0:p0 + 64, :])
                            nc.vector.tensor_mul(dst, dst, ps_bc[p0:p0 + 64, :])
                            if ph == 3 and 3 in phases:
                                emit_phase3(pq)

    nc.compile()
    return nc


def make_tables(s_n=S):
    """Host-side RoPE tables (pair-permuted layout) and causal maskT."""
    theta = (1.0 / (10000.0 ** (np.arange(0, HD, 2, dtype=np.float32) / HD))).astype(
        np.float32
    )
    freqs = np.arange(s_n, dtype=np.float32)[:, None] * theta[None, :]  # [s, 32]
    cos = np.cos(freqs).astype(np.float32)
    sin = np.sin(freqs).astype(np.float32)
    cosh = np.tile(cos, (1, DQK // HD))  # [s, 160] (5 groups of 32)
    sinh = np.tile(sin, (1, DQK // HD))
    a = np.arange(128)
    # sT layout: rows = k, cols = q; mask out k > q.
    maskadd = np.where(a[:, None] <= a[None, :], 0.0, MASK_NEG).astype(np.float32)
    return cosh, sinh, maskadd


def _bf16(x):
    import ml_dtypes
    return np.ascontiguousarray(x).astype(ml_dtypes.bfloat16)


# per-head feature permutation: evens then odds
_PERM = np.concatenate([np.arange(0, HD, 2), np.arange(1, HD, 2)])


def make_core_inputs(x2, wq, wk, wv, wo, core):
    """Per-core input dict (host-side sharding prep)."""
    cosh, sinh, maskadd = _TABLES
    i = core
    wq_i = wq[i * DQ:(i + 1) * DQ].reshape(HC, HD, E)[:, _PERM, :].reshape(DQ, E)
    wk_i = wk[i * DKV:(i + 1) * DKV][_PERM, :]
    wv_i = wv[i * DKV:(i + 1) * DKV]
    wt = np.concatenate([wq_i, wk_i, wv_i], axis=0).T
    wot = wo[:, i * DQ:(i + 1) * DQ].T
    return {
        "xt": _bf16(x2.T),
        "wt": _bf16(wt),
        "wot": _bf16(wot),
        "cosh": cosh,
        "sinh": sinh,
        "maskadd": maskadd,
        "ident": _bf16(np.eye(128, dtype=np.float32)),
    }


_TABLES = make_tables()
_NC_CACHE = {}


def _get_nc(reps=1):
    key = ("nc", reps)
    if key not in _NC_CACHE:
        _NC_CACHE[key] = build_nc(reps=reps)
    return _NC_CACHE[key]


def kernel(x, wq, wk, wv, wo):
    x = np.asarray(x, dtype=np.float32)
    b, s_n, e = x.shape
    x2 = np.ascontiguousarray(x.reshape(s_n, e))
    in_maps = [
        make_core_inputs(x2, np.asarray(wq, np.float32), np.asarray(wk, np.float32),
                         np.asarray(wv, np.float32), np.asarray(wo, np.float32), i)
        for i in range(NCORES)
    ]
    res = run_bass_kernel_spmd(_get_nc(), in_maps, core_ids=list(range(NCORES)))
    out = np.zeros((s_n, e), dtype=np.float32)
    for rr in res.results:
        out += rr["out"]
    return out.reshape(b, s_n, e).astype(np.float32)


# revision 11
# speedup vs baseline: 1.7840x; 1.1144x over previous
"""Causal self-attention (GQA, RoPE) Trainium2 Bass kernel.

Full inputs in, full output out. Tensor-parallel over heads across 8
NeuronCores: core i computes q-heads 4i..4i+3 (kv head i) and a partial
output projection over its 256 attn-out features; the host sums the 8
partial outputs (the "all-reduce after output_proj" step).

v2 design notes (vs the v1 kernel):
- x is transposed on the HOST and shipped as bf16 [E, S]; the qkv
  projection consumes it directly (no on-device x transposes).
- Attention is computed in transposed layout: sT[k,q] = kT.T @ qT and
  outT[d,q] = v.T @ exp(sT), so no per-chunk attention transposes and
  no PSUM-evacuate/reload chains. Softmax denominators come from a
  ones-column appended to v; normalization is one rank-1 broadcast
  matmul + one DVE multiply per (head, q-block).
- Per-head features are host-permuted to [evens, odds] so RoPE is
  contiguous elementwise in both layouts; v/wo stay unpermuted.
- All matmul operands are bf16 (FWL fast weight loads); PSUM stays
  fp32.
"""

import numpy as np

import concourse.bacc as bacc
import concourse.mybir as mybir
import concourse.tile as tile
from concourse.bass_utils import run_bass_kernel_spmd

S = 2048          # sequence length
E = 2048          # embedding dim
H = 32            # query heads
KV = 8            # kv heads
HD = 64           # head dim
NCORES = 8
HC = H // NCORES  # query heads per core = 4
DQ = HC * HD      # per-core q proj width = 256
DKV = HD          # per-core kv proj width = 64
DQK = DQ + DKV    # roped span = 320
DW = DQ + 2 * DKV  # fused qkv proj width = 384
ST = S // 128     # 16 s-tiles of 128 rows
MASK_NEG = -1.0e4  # pre-scale additive mask (scaled: -1250 -> exp == 0)

F32 = mybir.dt.float32
F32R = mybir.dt.float32r
BF16 = mybir.dt.bfloat16


def r(ap):
    """Bitcast an AP to float32r so the PE runs fast-mode fp32 matmuls."""
    return ap.bitcast(F32R)


def build_nc(seq_tiles=ST, reps=1, phases=(1, 2, 3)):
    """Build + compile the per-core Bass program (identical on all cores)."""
    st_n = seq_tiles
    s_n = st_n * 128
    nit = st_n  # (head, q-block) iterations: 4 heads x st_n/4 q-blocks

    nc = bacc.Bacc("TRN2", target_bir_lowering=False, debug=False)
    xt_d = nc.dram_tensor("xt", [E, s_n], BF16, kind="ExternalInput")
    wt_d = nc.dram_tensor("wt", [E, DW], BF16, kind="ExternalInput")
    wot_d = nc.dram_tensor("wot", [DQ, E], BF16, kind="ExternalInput")
    cos_d = nc.dram_tensor("cosh", [s_n, DQK // 2], F32, kind="ExternalInput")
    sin_d = nc.dram_tensor("sinh", [s_n, DQK // 2], F32, kind="ExternalInput")
    mask_d = nc.dram_tensor("maskadd", [128, 128], F32, kind="ExternalInput")
    id_d = nc.dram_tensor("ident", [128, 128], BF16, kind="ExternalInput")
    out_d = nc.dram_tensor("out", [s_n, E], BF16, kind="ExternalOutput")

    with tile.TileContext(nc) as tc:
        for _rep in range(reps):
            with (
                tc.tile_pool(name="const", bufs=1) as constp,
                tc.tile_pool(name="store", bufs=1) as storep,
            ):
                ident = constp.tile([128, 128], BF16)
                nc.sync.dma_start(out=ident[:], in_=id_d.ap()[:, :])
                maskT_sb = constp.tile([128, 128], F32)
                nc.sync.dma_start(out=maskT_sb[:], in_=mask_d.ap()[:, :])
                wT_sb = constp.tile([128, E // 128, DW], BF16)
                nc.sync.dma_start(
                    out=wT_sb[:], in_=wt_d.ap().rearrange("(c p) f -> p c f", p=128)
                )
                woT_sb = constp.tile([128, 2, E], BF16)
                nc.sync.dma_start(
                    out=woT_sb[:], in_=wot_d.ap().rearrange("(c p) e -> p c e", p=128)
                )
                ones_sb = constp.tile([128, 128], F32)
                nc.vector.memset(ones_sb[:], 1.0)

                # qT: head h cols [h, s]; kT: [s]; d on partitions 0:64.
                qT_sb = storep.tile([64, HC, s_n], BF16)
                kT_sb = storep.tile([64, s_n], BF16)
                # v in [s, d] per 128-chunk; _ev has a ones col at 64 (den row
                # 64 of AV psum), _od has ones col at 0 + v at 64:128.
                v_ev = storep.tile([128, st_n, 72], BF16)
                v_od = storep.tile([128, st_n, 128], BF16)
                nc.vector.memset(v_ev[:, :, 64:72], 1.0)
                nc.vector.memset(v_od[:, :, 0:1], 1.0)
                nc.vector.memset(v_od[:, :, 1:64], 0.0)
                # attn-out transposed: feature d = c*128 + p, col = s.
                aoT_sb = storep.tile([128, 2, s_n], BF16)

                # ================= phase 1: qkv proj + rope =================
                with (
                    tc.tile_pool(name="p1_x", bufs=2) as p1x,
                    tc.tile_pool(name="p1_sb", bufs=2) as p1,
                    tc.tile_pool(name="p1_ps_qkv", bufs=2, space="PSUM") as ps_qkv_p,
                    tc.tile_pool(name="p1_ps_tr", bufs=2, space="PSUM") as ps_tr_p,
                ):
                    xt_r = xt_d.ap().rearrange("(c p) s -> p c s", p=128)
                    xT_blk = [None] * (st_n // 4)
                    prev = None  # (qk_sb tile, ps_qkv, t)
                    tr_prev = None  # (ps_tr, t)

                    def emit_tr(qk_sb, t):
                        ps_tr = ps_tr_p.tile([64, 5, 128], BF16, tag="tr")
                        for g in range(5):
                            nc.tensor.matmul(
                                ps_tr[:, g, :],
                                qk_sb[:, g * 64:(g + 1) * 64],
                                ident[:],
                                is_transpose=True,
                                start=(g == 0),
                                stop=(g == 4),
                            )
                        return ps_tr

                    def emit_trcopy(ps_tr, t):
                        nc.vector.tensor_copy(
                            qT_sb[:, :, t * 128:(t + 1) * 128], ps_tr[:, 0:4, :]
                        )
                        nc.vector.tensor_copy(
                            kT_sb[:, t * 128:(t + 1) * 128], ps_tr[:, 4, :]
                        )

                    for t in range(st_n if 1 in phases else 0):
                        blk, ts = t // 4, t % 4
                        if ts == 0:
                            xT_blk[blk] = p1x.tile(
                                [128, E // 128, 512], BF16, tag="xT", name="xTb"
                            )
                            nc.sync.dma_start(
                                out=xT_blk[blk][:],
                                in_=xt_r[:, :, blk * 512:(blk + 1) * 512],
                            )
                        cs_sb = p1.tile([128, 2, DQK // 2], F32, tag="cs")
                        nc.sync.dma_start(
                            out=cs_sb[:, 0, :], in_=cos_d.ap()[t * 128:(t + 1) * 128, :]
                        )
                        nc.sync.dma_start(
                            out=cs_sb[:, 1, :], in_=sin_d.ap()[t * 128:(t + 1) * 128, :]
                        )
                        ps_qkv = ps_qkv_p.tile([128, DW], F32, tag="qkv")
                        for c in range(E // 128):
                            nc.tensor.matmul(
                                ps_qkv[:],
                                xT_blk[blk][:, c, ts * 128:(ts + 1) * 128],
                                wT_sb[:, c, :],
                                start=(c == 0),
                                stop=(c == E // 128 - 1),
                            )
                        if tr_prev is not None:
                            emit_trcopy(*tr_prev)
                            tr_prev = None
                        if prev is not None:
                            tr_prev = (emit_tr(*prev), prev[1])

                        # ---- rope on q+k jointly (5 groups of [ev|od]) ----
                        se = ps_qkv[:, 0:DQK].rearrange(
                            "p (g two d) -> p two g d", two=2, d=32
                        )
                        qk_sb = p1.tile([128, DQK], BF16, tag="qkro")
                        de = qk_sb[:].rearrange("p (g two d) -> p two g d", two=2, d=32)
                        c_ap = cs_sb[:, 0, :].rearrange("p (g d) -> p g d", d=32)
                        s_ap = cs_sb[:, 1, :].rearrange("p (g d) -> p g d", d=32)
                        t1 = p1.tile([128, DQK // 2], F32, tag="t1")
                        t2 = p1.tile([128, DQK // 2], F32, tag="t2")
                        nc.vector.tensor_mul(t1[:], se[:, 0, :, :], c_ap)
                        nc.vector.tensor_mul(t2[:], se[:, 1, :, :], s_ap)
                        nc.vector.tensor_sub(de[:, 0, :, :], t1[:], t2[:])
                        t3 = p1.tile([128, DQK // 2], F32, tag="t3")
                        t4 = p1.tile([128, DQK // 2], F32, tag="t4")
                        nc.vector.tensor_mul(t3[:], se[:, 1, :, :], c_ap)
                        nc.vector.tensor_mul(t4[:], se[:, 0, :, :], s_ap)
                        nc.vector.tensor_add(de[:, 1, :, :], t3[:], t4[:])
                        # ---- v copies (both AV lhsT layouts) ----
                        nc.vector.tensor_copy(v_ev[:, t, 0:64], ps_qkv[:, DQK:DW])
                        nc.vector.tensor_copy(v_od[:, t, 64:128], ps_qkv[:, DQK:DW])
                        prev = (qk_sb, t)

                    if tr_prev is not None:
                        emit_trcopy(*tr_prev)
                    if prev is not None:
                        ps_tr = emit_tr(*prev)
                        emit_trcopy(ps_tr, prev[1])

                # ============ phase 2+3: attention + output proj ============
                with (
                    tc.tile_pool(name="p2_aT", bufs=2) as p2a,
                    tc.tile_pool(name="p2_rt", bufs=2) as p2r,
                    tc.tile_pool(name="p3_o", bufs=2) as p3o,
                    tc.tile_pool(name="p2_ps_s", bufs=3, space="PSUM") as ps_s_p,
                    tc.tile_pool(name="p2_ps_av", bufs=2, space="PSUM") as ps_av_p,
                    tc.tile_pool(name="p2_ps_bc", bufs=1, space="PSUM") as ps_bc_p,
                    tc.tile_pool(name="p3_ps", bufs=2, space="PSUM") as ps_o_p,
                ):
                    qb_n = st_n // 4

                    def emit_phase3(qb):
                        for st in range(4 * qb, 4 * qb + 4):
                            o_sb = p3o.tile([128, E], BF16, tag="o")
                            for eb in range(E // 512):
                                ps_o = ps_o_p.tile([128, 512], F32, tag="po")
                                for c2 in range(2):
                                    nc.tensor.matmul(
                                        ps_o[:],
                                        aoT_sb[:, c2, st * 128:(st + 1) * 128],
                                        woT_sb[:, c2, eb * 512:(eb + 1) * 512],
                                        start=(c2 == 0),
                                        stop=(c2 == 1),
                                    )
                                if eb % 2 == 0:
                                    nc.vector.tensor_copy(
                                        o_sb[:, eb * 512:(eb + 1) * 512], ps_o[:]
                                    )
                                else:
                                    nc.scalar.copy(
                                        o_sb[:, eb * 512:(eb + 1) * 512], ps_o[:]
                                    )
                            nc.gpsimd.dma_start(
                                out=out_d.ap()[st * 128:(st + 1) * 128, :], in_=o_sb[:]
                            )

                    aT_t = {}
                    av_t = {}
                    for it in range(nit + 1 if 2 in phases else 0):
                        cur = it if it < nit else None
                        prv = it - 1 if it > 0 else None
                        if cur is not None:
                            qb, h = divmod(cur, 4)
                            k_cur = 4 * qb + 4
                            aT = p2a.tile([128, st_n, 512], BF16, tag="aT")
                            aT_t[cur] = aT
                        if prv is not None:
                            pq, ph = divmod(prv, 4)
                            k_prv = 4 * pq + 4
                            ps_av = ps_av_p.tile([128, 512], F32, tag="av")
                            av_t[prv] = ps_av
                            pT = aT_t[prv]
                        nk = max(k_cur if cur is not None else 0,
                                 k_prv if prv is not None else 0)
                        for kc in range(nk):
                            if cur is not None and kc < k_cur:
                                lo = 128 * max(0, kc - 4 * qb)
                                ps_s = ps_s_p.tile([128, 512], F32, tag="s")
                                nc.tensor.matmul(
                                    ps_s[:, lo:512],
                                    kT_sb[:, kc * 128:(kc + 1) * 128],
                                    qT_sb[:, h, qb * 512 + lo:(qb + 1) * 512],
                                    start=True,
                                    stop=True,
                                )
                                if kc >= 4 * qb:  # diagonal chunk: causal mask
                                    nc.vector.tensor_add(
                                        ps_s[:, lo:lo + 128],
                                        ps_s[:, lo:lo + 128],
                                        maskT_sb[:],
                                    )
                                nc.scalar.activation(
                                    aT[:, kc, lo:512],
                                    ps_s[:, lo:512],
                                    mybir.ActivationFunctionType.Exp,
                                    scale=0.125,
                                )
                            if prv is not None and kc < k_prv:
                                lo = 128 * max(0, kc - 4 * pq)
                                vt = (v_ev[:, kc, 0:65] if ph % 2 == 0
                                      else v_od[:, kc, :])
                                np_out = 65 if ph % 2 == 0 else 128
                                nc.tensor.matmul(
                                    ps_av[0:np_out, lo:512],
                                    vt,
                                    pT[:, kc, lo:512],
                                    start=(kc == 0),
                                    stop=(kc == k_prv - 1),
                                )
                        if prv is not None:
                            # normalize: aoT = v-out rows * (1/den) bcast
                            dr = 64 if ph % 2 == 0 else 0
                            # approx-recip is broken for base_partition != 0 /
                            # PSUM reads: bounce den to SBUF (ACT), recip rows
                            # [0:dr+1] from base 0, round to f32r (ACT).
                            den_sb = p2r.tile([128, 512], F32, tag="dn")
                            nc.scalar.copy(den_sb[dr:dr + 1, :], ps_av[dr:dr + 1, :])
                            rtmp = p2r.tile([128, 512], F32, tag="rt")
                            nc.vector.reciprocal_approx_fast(
                                rtmp[0:dr + 1, :], den_sb[0:dr + 1, :]
                            )
                            rinv = p2r.tile([128, 512], F32R, tag="ri")
                            nc.scalar.copy(rinv[dr:dr + 1, :], rtmp[dr:dr + 1, :])
                            ps_bc = ps_bc_p.tile([128, 512], F32, tag="bc")
                            nc.tensor.matmul(
                                ps_bc[:],
                                r(ones_sb[dr:dr + 1, :]),
                                rinv[dr:dr + 1, :],
                                start=True,
                                stop=True,
                            )
                            p0 = 64 * (ph & 1)
                            hp = ph >> 1
                            dst = aoT_sb[p0:p0 + 64, hp, pq * 512:(pq + 1) * 512]
                            nc.vector.tensor_copy(dst, ps_av[p0:p0 + 64, :])
                            nc.vector.tensor_mul(dst, dst, ps_bc[p0:p0 + 64, :])
                            if ph == 3 and 3 in phases:
                                emit_phase3(pq)

    nc.compile()
    return nc


def make_tables(s_n=S):
    """Host-side RoPE tables (pair-permuted layout) and causal maskT."""
    theta = (1.0 / (10000.0 ** (np.arange(0, HD, 2, dtype=np.float32) / HD))).astype(
        np.float32
    )
    freqs = np.arange(s_n, dtype=np.float32)[:, None] * theta[None, :]  # [s, 32]
    cos = np.cos(freqs).astype(np.float32)
    sin = np.sin(freqs).astype(np.float32)
    cosh = np.tile(cos, (1, DQK // HD))  # [s, 160] (5 groups of 32)
    sinh = np.tile(sin, (1, DQK // HD))
    a = np.arange(128)
    # sT layout: rows = k, cols = q; mask out k > q.
    maskadd = np.where(a[:, None] <= a[None, :], 0.0, MASK_NEG).astype(np.float32)
    return cosh, sinh, maskadd


def _bf16(x):
    import ml_dtypes
    return np.ascontiguousarray(x).astype(ml_dtypes.bfloat16)


# per-head feature permutation: evens then odds
_PERM = np.concatenate([np.arange(0, HD, 2), np.arange(1, HD, 2)])


def make_core_inputs(x2, wq, wk, wv, wo, core):
    """Per-core input dict (host-side sharding prep)."""
    cosh, sinh, maskadd = _TABLES
    i = core
    wq_i = wq[i * DQ:(i + 1) * DQ].reshape(HC, HD, E)[:, _PERM, :].reshape(DQ, E)
    wk_i = wk[i * DKV:(i + 1) * DKV][_PERM, :]
    wv_i = wv[i * DKV:(i + 1) * DKV]
    wt = np.concatenate([wq_i, wk_i, wv_i], axis=0).T
    wot = wo[:, i * DQ:(i + 1) * DQ].T
    return {
        "xt": _bf16(x2.T),
        "wt": _bf16(wt),
        "wot": _bf16(wot),
        "cosh": cosh,
        "sinh": sinh,
        "maskadd": maskadd,
        "ident": _bf16(np.eye(128, dtype=np.float32)),
    }


_TABLES = make_tables()
_NC_CACHE = {}


def _get_nc(reps=1):
    key = ("nc", reps)
    if key not in _NC_CACHE:
        _NC_CACHE[key] = build_nc(reps=reps)
    return _NC_CACHE[key]


def kernel(x, wq, wk, wv, wo):
    x = np.asarray(x, dtype=np.float32)
    b, s_n, e = x.shape
    x2 = np.ascontiguousarray(x.reshape(s_n, e))
    in_maps = [
        make_core_inputs(x2, np.asarray(wq, np.float32), np.asarray(wk, np.float32),
                         np.asarray(wv, np.float32), np.asarray(wo, np.float32), i)
        for i in range(NCORES)
    ]
    res = run_bass_kernel_spmd(_get_nc(), in_maps, core_ids=list(range(NCORES)))
    out = np.zeros((s_n, e), dtype=np.float32)
    for rr in res.results:
        out += np.asarray(rr["out"]).astype(np.float32)
    return out.reshape(b, s_n, e).astype(np.float32)
